# revision 2
# baseline (speedup 1.0000x reference)
import math
import numpy as np

HIDDEN = 768
HEADS = 12
HEAD_DIM = HIDDEN // HEADS  # 64
NUM_BUCKETS = 32
MAX_DIST = 128
EPS = 1e-6

# Problem shape (hardcoded per spec): x is (T,B,C,H,W,D) = (16,1,768,16,16,8)
T, B, C, H, W, D = 16, 1, 768, 16, 16, 8
M_CORES = 8
HS = H // M_CORES  # 2 h-planes per core: pure data parallelism over spatial axis


def _rel_buckets(Tn):
    # T5 bidirectional relative-position bucketing (static index table).
    ctx = np.arange(Tn)[:, None]
    mem = np.arange(Tn)[None, :]
    rp = mem - ctx
    nb = NUM_BUCKETS // 2
    ret = (rp > 0).astype(np.int64) * nb
    n = np.abs(rp)
    max_exact = nb // 2
    is_small = n < max_exact
    val_large = max_exact + (
        np.log(np.maximum(n, 1) / max_exact)
        / math.log(MAX_DIST / max_exact)
        * (nb - max_exact)
    ).astype(np.int64)
    val_large = np.minimum(val_large, nb - 1)
    return ret + np.where(is_small, n, val_large)


_BUCKETS = _rel_buckets(T)
_CACHE = {}


def _get_compiled():
    if "fns" in _CACHE:
        return _CACHE["fns"]
    import jax
    import jax.numpy as jnp

    try:
        jax.config.update("jax_compilation_cache_dir", "/tmp/jax_cache_axial")
        jax.config.update("jax_persistent_cache_min_entry_size_bytes", -1)
        jax.config.update("jax_persistent_cache_min_compile_time_secs", 0.0)
    except Exception:
        pass

    devs = jax.devices()[:M_CORES]
    assert len(devs) == M_CORES
    bf16 = jnp.bfloat16
    scale = 1.0 / math.sqrt(HEAD_DIM)

    # Stage 1: per-shard partial sum-of-squares for the RMS group norm.
    # Returns xs unchanged so the shard stays device-resident for stage 2
    # (avoids a second 100 MB host->device transfer).
    def fn1(xs):
        xg = xs.reshape(T * B, HEADS, C // HEADS, HS, W, D)
        return jnp.sum(xg * xg, axis=(2, 3, 4, 5)), xs

    # Stage 2: full forward for one spatial shard given the global ssq.
    # All large matmuls/einsums run with bf16 operands and fp32 accumulation
    # (fp32 matmul is 4x slower on the PE array); norms/softmax stay fp32.
    def fn2(xs, ssq, w_, w_inT, b_in_, qs_, qb_, ks_, kb_, bias_, w_outT, b_out_):
        ms = ssq / float((C // HEADS) * H * W * D)
        inv = jax.lax.rsqrt(ms + EPS)  # (T*B, HEADS)
        xg = xs.reshape(T * B, HEADS, C // HEADS, HS, W, D)
        xn = (xg * inv[:, :, None, None, None, None]).reshape(T * B, C, HS, W, D)
        xn = xn * w_[None, :, None, None, None]
        xt = jnp.transpose(xn, (0, 2, 3, 4, 1)).astype(bf16)  # (TB,HS,W,D,C)
        qkv = jax.lax.dot_general(
            xt.reshape(-1, C),
            w_inT.astype(bf16),
            (((1,), (0,)), ((), ())),
            preferred_element_type=jnp.float32,
        ) + b_in_  # (TB*HS*W*D, 3C) fp32
        qkv = qkv.reshape(T, B, HS, W, D, HEADS, 3 * HEAD_DIM)
        qkv = jnp.transpose(qkv, (1, 2, 3, 4, 5, 0, 6))
        qkv = qkv.reshape(B * HS * W * D, HEADS, T, 3 * HEAD_DIM)
        q, k, v = jnp.split(qkv, 3, axis=-1)

        def ln(t, sc, bi):
            mu = jnp.mean(t, axis=-1, keepdims=True)
            var = jnp.mean((t - mu) ** 2, axis=-1, keepdims=True)
            return (t - mu) * jax.lax.rsqrt(var + EPS) * sc + bi

        q = ln(q, qs_, qb_).astype(bf16)
        k = ln(k, ks_, kb_).astype(bf16)
        logits = (
            jnp.einsum("bhsc,bhtc->bhst", q, k, preferred_element_type=jnp.float32)
            * scale
            + bias_
        )
        attn = jax.nn.softmax(logits, axis=-1).astype(bf16)
        out = jnp.einsum(
            "bhst,bhtc->bhsc", attn, v.astype(bf16),
            preferred_element_type=jnp.float32,
        )
        out = out.reshape(B, HS, W, D, HEADS, T, HEAD_DIM)
        out = jnp.transpose(out, (5, 0, 4, 6, 1, 2, 3)).reshape(T * B, C, HS, W, D)
        ot_in = jnp.transpose(out, (0, 2, 3, 4, 1)).astype(bf16)
        ot = jax.lax.dot_general(
            ot_in.reshape(-1, C),
            w_outT.astype(bf16),
            (((1,), (0,)), ((), ())),
            preferred_element_type=jnp.float32,
        ) + b_out_
        ot = ot.reshape(T * B, HS, W, D, C)
        y = jnp.transpose(ot, (0, 4, 1, 2, 3)).reshape(T, B, C, HS, W, D)
        return y + xs

    p1 = jax.pmap(fn1, devices=devs)
    p2 = jax.pmap(fn2, devices=devs, in_axes=(0,) + (None,) * 11)

    _CACHE["fns"] = (jax, jnp, devs, p1, p2)
    return _CACHE["fns"]


def kernel(**inputs):
    jax, jnp, devs, p1, p2 = _get_compiled()

    x = np.asarray(inputs["x"], np.float32)
    w_norm = np.asarray(inputs["norm1_weight"], np.float32)
    w_in = np.asarray(inputs["input_head_weight"], np.float32)[:, :, 0, 0, 0]
    b_in = np.asarray(inputs["input_head_bias"], np.float32)
    q_s = np.asarray(inputs["qnorm_scale"], np.float32)
    q_b = np.asarray(inputs["qnorm_bias"], np.float32)
    k_s = np.asarray(inputs["knorm_scale"], np.float32)
    k_b = np.asarray(inputs["knorm_bias"], np.float32)
    rbt = np.asarray(inputs["rel_bias_table"], np.float32)
    w_out = np.asarray(inputs["output_head_weight"], np.float32)[:, :, 0, 0, 0]
    b_out = np.asarray(inputs["output_head_bias"], np.float32)

    # Precompute the (1, He, T, T) additive attention bias host-side.
    bias = np.ascontiguousarray(
        rbt[_BUCKETS].transpose(2, 0, 1)[None]
    ).astype(np.float32)

    # Shard the spatial H axis across the 8 cores.
    xs_stack = np.stack(
        [x[:, :, :, i * HS : (i + 1) * HS] for i in range(M_CORES)], axis=0
    )  # (8,T,B,C,HS,W,D)

    ssq_parts, xs_dev = p1(xs_stack)
    ssq = np.asarray(ssq_parts).sum(axis=0)  # combine tiny partials (192 floats)

    y_stack = p2(
        xs_dev,
        ssq,
        w_norm,
        w_in.T.copy(),
        b_in,
        q_s,
        q_b,
        k_s,
        k_b,
        bias,
        w_out.T.copy(),
        b_out,
    )
    y_stack = np.asarray(y_stack)  # (8,T,B,C,HS,W,D)
    y = np.concatenate([y_stack[i] for i in range(M_CORES)], axis=3)
    return y.astype(np.float32)


# revision 7
# speedup vs baseline: 1.9981x; 1.9981x over previous
import math
import numpy as np

HIDDEN = 768
HEADS = 12
HEAD_DIM = HIDDEN // HEADS  # 64
NUM_BUCKETS = 32
MAX_DIST = 128
EPS = 1e-6

# Problem shape (hardcoded per spec): x is (T,B,C,H,W,D) = (16,1,768,16,16,8)
T, B, C, H, W, D = 16, 1, 768, 16, 16, 8
M_CORES = 8
HS = H // M_CORES  # 2 h-planes per core: pure data parallelism over spatial axis


def _rel_buckets(Tn):
    # T5 bidirectional relative-position bucketing (static index table).
    ctx = np.arange(Tn)[:, None]
    mem = np.arange(Tn)[None, :]
    rp = mem - ctx
    nb = NUM_BUCKETS // 2
    ret = (rp > 0).astype(np.int64) * nb
    n = np.abs(rp)
    max_exact = nb // 2
    is_small = n < max_exact
    val_large = max_exact + (
        np.log(np.maximum(n, 1) / max_exact)
        / math.log(MAX_DIST / max_exact)
        * (nb - max_exact)
    ).astype(np.int64)
    val_large = np.minimum(val_large, nb - 1)
    return ret + np.where(is_small, n, val_large)


_BUCKETS = _rel_buckets(T)
_CACHE = {}


def _get_compiled():
    if "fns" in _CACHE:
        return _CACHE["fns"]
    import jax
    import jax.numpy as jnp

    try:
        jax.config.update("jax_compilation_cache_dir", "/tmp/jax_cache_axial")
        jax.config.update("jax_persistent_cache_min_entry_size_bytes", -1)
        jax.config.update("jax_persistent_cache_min_compile_time_secs", 0.0)
    except Exception:
        pass

    devs = jax.devices()[:M_CORES]
    assert len(devs) == M_CORES
    bf16 = jnp.bfloat16
    scale = 1.0 / math.sqrt(HEAD_DIM)

    # Stage 1: per-shard partial sum-of-squares for the RMS group norm.
    # Input arrives in bf16 (halves the host->device transfer); stats are
    # accumulated in fp32. Returns xb unchanged so the shard stays
    # device-resident for stage 2.
    def fn1(xb):
        xf = xb.astype(jnp.float32).reshape(T * B, HEADS, C // HEADS, HS, W, D)
        return jnp.sum(xf * xf, axis=(2, 3, 4, 5)), xb

    # Stage 2: full forward for one spatial shard given the global ssq.
    # All large matmuls/einsums run with bf16 operands and fp32 accumulation
    # (fp32 matmul is 4x slower on the PE array); norms/softmax stay fp32.
    def fn2(xb, ssq, w_, w_inT, b_in_, qs_, qb_, ks_, kb_, bias_, w_outT, b_out_):
        ms = ssq / float((C // HEADS) * H * W * D)
        inv = jax.lax.rsqrt(ms + EPS)  # (T*B, HEADS)
        xg = xb.astype(jnp.float32).reshape(T * B, HEADS, C // HEADS, HS, W, D)
        xn = (xg * inv[:, :, None, None, None, None]).reshape(T * B, C, HS, W, D)
        xn = xn * w_[None, :, None, None, None]
        xt = jnp.transpose(xn, (0, 2, 3, 4, 1)).astype(bf16)  # (TB,HS,W,D,C)
        qkv = jax.lax.dot_general(
            xt.reshape(-1, C),
            w_inT.astype(bf16),
            (((1,), (0,)), ((), ())),
            preferred_element_type=jnp.float32,
        ) + b_in_  # (TB*HS*W*D, 3C) fp32
        qkv = qkv.reshape(T, B, HS, W, D, HEADS, 3 * HEAD_DIM)
        qkv = jnp.transpose(qkv, (1, 2, 3, 4, 5, 0, 6))
        qkv = qkv.reshape(B * HS * W * D, HEADS, T, 3 * HEAD_DIM)
        q, k, v = jnp.split(qkv, 3, axis=-1)

        def ln(t, sc, bi):
            mu = jnp.mean(t, axis=-1, keepdims=True)
            var = jnp.mean((t - mu) ** 2, axis=-1, keepdims=True)
            return (t - mu) * jax.lax.rsqrt(var + EPS) * sc + bi

        q = ln(q, qs_, qb_).astype(bf16)
        k = ln(k, ks_, kb_).astype(bf16)
        logits = (
            jnp.einsum("bhsc,bhtc->bhst", q, k, preferred_element_type=jnp.float32)
            * scale
            + bias_
        )
        attn = jax.nn.softmax(logits, axis=-1).astype(bf16)
        out = jnp.einsum(
            "bhst,bhtc->bhsc", attn, v.astype(bf16),
            preferred_element_type=jnp.float32,
        )
        out = out.reshape(B, HS, W, D, HEADS, T, HEAD_DIM)
        out = jnp.transpose(out, (5, 0, 4, 6, 1, 2, 3)).reshape(T * B, C, HS, W, D)
        ot_in = jnp.transpose(out, (0, 2, 3, 4, 1)).astype(bf16)
        ot = jax.lax.dot_general(
            ot_in.reshape(-1, C),
            w_outT.astype(bf16),
            (((1,), (0,)), ((), ())),
            preferred_element_type=jnp.float32,
        ) + b_out_
        ot = ot.reshape(T * B, HS, W, D, C)
        y = jnp.transpose(ot, (0, 4, 1, 2, 3)).reshape(T, B, C, HS, W, D)
        # Residual is added host-side in fp32; ship the attention output
        # down in bf16 (halves the device->host transfer).
        return y.astype(bf16)

    p1 = jax.pmap(fn1, devices=devs)
    p2 = jax.pmap(fn2, devices=devs, in_axes=(0,) + (None,) * 11)

    _CACHE["fns"] = (jax, jnp, devs, p1, p2)
    return _CACHE["fns"]


def kernel(**inputs):
    jax, jnp, devs, p1, p2 = _get_compiled()

    x = np.asarray(inputs["x"], np.float32)
    w_norm = np.asarray(inputs["norm1_weight"], np.float32)
    w_in = np.asarray(inputs["input_head_weight"], np.float32)[:, :, 0, 0, 0]
    b_in = np.asarray(inputs["input_head_bias"], np.float32)
    q_s = np.asarray(inputs["qnorm_scale"], np.float32)
    q_b = np.asarray(inputs["qnorm_bias"], np.float32)
    k_s = np.asarray(inputs["knorm_scale"], np.float32)
    k_b = np.asarray(inputs["knorm_bias"], np.float32)
    rbt = np.asarray(inputs["rel_bias_table"], np.float32)
    w_out = np.asarray(inputs["output_head_weight"], np.float32)[:, :, 0, 0, 0]
    b_out = np.asarray(inputs["output_head_bias"], np.float32)

    # Precompute the (1, He, T, T) additive attention bias host-side.
    bias = np.ascontiguousarray(
        rbt[_BUCKETS].transpose(2, 0, 1)[None]
    ).astype(np.float32)

    import ml_dtypes

    # Shard the spatial H axis across the 8 cores; upload in bf16.
    x_bf = x.astype(ml_dtypes.bfloat16)
    xs_stack = np.stack(
        [x_bf[:, :, :, i * HS : (i + 1) * HS] for i in range(M_CORES)], axis=0
    )  # (8,T,B,C,HS,W,D) bf16

    ssq_parts, xs_dev = p1(xs_stack)
    ssq = np.asarray(ssq_parts).sum(axis=0)  # combine tiny partials (192 floats)

    y_stack = p2(
        xs_dev,
        ssq,
        w_norm,
        w_in.T.copy(),
        b_in,
        q_s,
        q_b,
        k_s,
        k_b,
        bias,
        w_out.T.copy(),
        b_out,
    )

    # Fetch the 8 output shards concurrently (the tunnel serializes a single
    # np.asarray of the whole sharded array); each thread also does the fp32
    # upcast + residual add for its shard so host math overlaps the other
    # shards' transfers.
    from concurrent.futures import ThreadPoolExecutor

    shards = y_stack.addressable_shards
    y = np.empty((T, B, C, H, W, D), np.float32)

    def _fetch(i):
        part = np.asarray(shards[i].data).reshape(T, B, C, HS, W, D)
        sl = slice(i * HS, (i + 1) * HS)
        np.add(
            part.astype(np.float32), x[:, :, :, sl], out=y[:, :, :, sl]
        )

    with ThreadPoolExecutor(M_CORES) as ex:
        list(ex.map(_fetch, range(M_CORES)))
    return y
